# revision 7
# baseline (speedup 1.0000x reference)
"""Cox partial-likelihood loss on 8 Trainium2 NeuronCores.

reference:
    theta = hazard_pred.reshape(-1)                 # [n]
    R[i, j] = survtime[j] >= survtime[i]            # risk-set mask
    risk_sum[i] = sum_j exp(theta[j]) * R[i, j]
    loss = -mean((theta - log(risk_sum)) * censor)

Sharding: rows i are split across 8 cores (1024 rows each). Each core
computes its [8192 x 1024] slice of the e_j-weighted risk mask in 64
chunks of 128 j's and contracts each chunk on the TensorEngine into
PSUM. j-index mapping: j = p*64 + c (p = SBUF partition, c = chunk).

v4 design — measured HW rates (ns per [128x1024] chunk):
    DVE is_le*e -> fp16 648, -> fp8 859 ; ACT Sign 1208
    PE plain [1,512]x2 ~640 ; fp8 DoubleRow pair ~432/chunk with a
    CONSTANT stationary, +240/matmul when the stationary changes
    (the 256-column DoubleRow ldweights cannot be hidden).
So: keep every DoubleRow stationary constant, and give the weighted
(varying) stationaries only to cheap 1-column plain ldweights:
  - NPAIR=18 DVE-DVE fp8 pairs (chunk ids 0..35): both subrows are
    DVE is_le*e fp8 chunks, stationary = constant fp8 ones pair.
  - NACT=28 ACT Sign fp16 chunks (ids 36..63): plain matmuls with a
    per-chunk h16[:,c] = fp16(e/2) [128,1] stationary.
All three lanes land at ~31-34us (ACT/PE binding, DVE ~31).

Survtime is fp16-rounded on host (fp32-stored for the compare scalar),
so diagonal self-compares tie exactly and is_le keeps the self-term.

Sign algebra for ACT chunks (A = chunks 36..63): PSUM accumulates
Sum_A h_j*sign(s_j - s_i) = 2*Sum_A h_j R_ij - S_h - h_i*[i in A], so
risk = P + S_h + h_i*[i in A]. S_h = Sum_A h_j folds into the Ln bias;
the h_i*w row is added into PSUM by a K=1 matmul (w from host, 0.5 on
ACT rows, times exp(theta_local) on device). fp16/fp8 rounding and tie
handling perturb the loss ~1e-5..1e-4 relative (validated vs the fp32
reference in numpy), well inside the 2e-2 gate.

ACT op order keeps every activation in the natural_log_exp table (exp,
sign, ln, copy) with at most one early table load: both Exp ops and an
Ln warm-up run before the Sign stream; Ln/Copy at the tail hit a warm
table.

Host sums the 8 partial row-sums and applies -1/n.
"""

import sys
from contextlib import ExitStack, nullcontext

import numpy as np

try:  # concourse ships with the container toolchain, not on sys.path by default
    import concourse  # noqa: F401
except ImportError:
    sys.path.insert(0, "/opt/trn_rl_repo")

import concourse.bacc as bacc
import concourse.bass as bass
import concourse.tile as tile
from concourse import mybir
from concourse.bass_utils import run_bass_kernel_spmd

DT = mybir.dt
AF = mybir.ActivationFunctionType
N = 8192
CORES = 8
NL = N // CORES       # 1024 local rows per core
NCHUNK = 64           # j-chunks of 128
NHALF = NL // 2       # matmul free-dim limit is 512 (PSUM bank)

NPAIR = 18                       # DVE-DVE fp8 DoubleRow pairs (tunable)
MASK_BUFS = 4


def _splits():
    nact = NCHUNK - 2 * NPAIR
    return nact, 2 * NPAIR

_CACHE: dict = {}


def _emit_body(nc, const, masks, psums, tailp, st32r_all, th_all, st16_loc,
               th_loc, cen_loc, wv, partial):
    NACT, ACT_LO = _splits()
    st32r = const.tile([128, NCHUNK], DT.float32)
    nc.sync.dma_start(out=st32r, in_=st32r_all[:].rearrange("(p c) -> p c", c=NCHUNK))
    th_sb = const.tile([128, NCHUNK], DT.float32)
    nc.sync.dma_start(out=th_sb, in_=th_all[:].rearrange("(p c) -> p c", c=NCHUNK))

    # tail inputs early (their ACT/gpsimd compute must beat the Sign stream)
    thl = tailp.tile([1, NL], DT.float32)
    nc.sync.dma_start(out=thl, in_=th_loc[:].rearrange("(o n) -> o n", o=1))
    cenl = tailp.tile([1, NL], DT.float32)
    nc.sync.dma_start(out=cenl, in_=cen_loc[:].rearrange("(o n) -> o n", o=1))
    wvl = tailp.tile([1, NL], DT.float32)
    nc.sync.dma_start(out=wvl, in_=wv[:].rearrange("(o n) -> o n", o=1))

    e32 = const.tile([128, NCHUNK], DT.float32)
    nc.scalar.activation(out=e32, in_=th_sb, func=AF.Exp)
    el32 = tailp.tile([1, NL], DT.float32)
    nc.scalar.activation(out=el32, in_=thl, func=AF.Exp)
    # settle the activation table (needs ln+exp+sign+copy => natural_log_exp
    # set) before the Sign stream so no mid-stream table swap occurs
    onesf = tailp.tile([1, 1], DT.float32)
    nc.vector.memset(onesf, 1.0)
    ln_warm = tailp.tile([1, 1], DT.float32)
    nc.scalar.activation(out=ln_warm, in_=onesf, func=AF.Ln)

    # local survtime (fp16) broadcast to all partitions (free dim = row i)
    si16 = const.tile([128, NL], DT.float16)
    st_loc_row = st16_loc[:].rearrange("(o n) -> o n", o=1)
    for q in range(4):
        nc.sync.dma_start(
            out=si16[q * 32 : (q + 1) * 32, :],
            in_=st_loc_row.partition_broadcast(32),
        )

    ones16_1 = const.tile([1, 1], DT.float16)
    nc.vector.memset(ones16_1, 1.0)
    ones8t = const.tile([128, 2, 16], DT.float8e4)
    nc.vector.memset(ones8t, 1.0)
    ones8 = ones8t[:, :, 0:1]

    # h16[:, c] = fp16(e_j/2) stationary columns for the ACT Sign chunks
    h32 = const.tile([128, NCHUNK], DT.float32)
    nc.vector.tensor_scalar(
        out=h32[:, ACT_LO:NCHUNK], in0=e32[:, ACT_LO:NCHUNK], scalar1=0.5,
        scalar2=None, op0=mybir.AluOpType.mult,
    )
    h16 = const.tile([128, NCHUNK], DT.float16)
    nc.vector.tensor_copy(out=h16[:, ACT_LO:NCHUNK], in_=h32[:, ACT_LO:NCHUNK])

    # S_h = sum over ACT-chunk j of the fp16-rounded h values
    hs32 = const.tile([128, NACT], DT.float32)
    nc.vector.tensor_copy(out=hs32, in_=h16[:, ACT_LO:NCHUNK])
    colsum = const.tile([128, 1], DT.float32)
    nc.vector.tensor_reduce(
        out=colsum, in_=hs32, axis=mybir.AxisListType.X, op=mybir.AluOpType.add
    )
    ones32 = const.tile([128, 1], DT.float32)
    nc.vector.memset(ones32, 1.0)
    psA = psums.tile([1, 1], DT.float32, tag="psa")
    nc.tensor.matmul(psA, ones32, colsum, start=True, stop=True)
    sA = const.tile([1, 1], DT.float32)
    nc.scalar.activation(out=sA, in_=psA, func=AF.Copy)

    # main stream: Bresenham-interleave DVE-DVE pairs among the ACT chunks
    p0 = psums.tile([1, NHALF], DT.float32, tag="p0")
    p1 = psums.tile([1, NHALF], DT.float32, tag="p1")
    seq = []
    emitted = 0
    acc = 0.0
    for t in range(NACT):
        acc += NPAIR / NACT
        while emitted < int(acc):
            seq.append(("pair", emitted))
            emitted += 1
        seq.append(("act", t))
    while emitted < NPAIR:
        seq.append(("pair", emitted))
        emitted += 1

    first = True
    for kind, idx in seq:
        if kind == "act":
            c = ACT_LO + idx
            m = masks.tile([128, NL], DT.float16, tag="ma")
            nc.scalar.activation(
                out=m, in_=si16, func=AF.Sign,
                bias=st32r[:, c : c + 1], scale=-1.0,
            )
            stat = h16[:, c : c + 1]
            nc.tensor.matmul(p0, stat, m[:, 0:NHALF], start=first, stop=False)
            nc.tensor.matmul(p1, stat, m[:, NHALF:NL], start=first, stop=False)
        else:
            c0 = 2 * idx
            c1 = 2 * idx + 1
            mp = masks.tile([128, 2, NL], DT.float8e4, tag="mp")
            for g, c in ((0, c0), (1, c1)):
                nc.vector.tensor_scalar(
                    out=mp[:, g, :], in0=si16, scalar1=st32r[:, c : c + 1],
                    scalar2=e32[:, c : c + 1], op0=mybir.AluOpType.is_le,
                    op1=mybir.AluOpType.mult,
                )
            nc.tensor.matmul(
                p0, ones8, mp[:, :, 0:NHALF], start=first, stop=False,
                perf_mode=mybir.MatmulPerfMode.DoubleRow,
            )
            nc.tensor.matmul(
                p1, ones8, mp[:, :, NHALF:NL], start=first, stop=False,
                perf_mode=mybir.MatmulPerfMode.DoubleRow,
            )
        first = False

    # theta*censor reduction off the critical path
    thc = tailp.tile([1, NL], DT.float32)
    nc.gpsimd.tensor_mul(thc, thl, cenl)
    thc_sum = tailp.tile([1, 1], DT.float32)
    nc.vector.tensor_reduce(
        out=thc_sum, in_=thc, axis=mybir.AxisListType.X, op=mybir.AluOpType.add
    )

    # diagonal Sign correction row: h_i on ACT rows (wv = 0.5 there, 0 else)
    corr32 = tailp.tile([1, NL], DT.float32)
    nc.gpsimd.tensor_mul(corr32, el32, wvl)
    corr16 = tailp.tile([1, NL], DT.float16)
    nc.gpsimd.tensor_copy(out=corr16, in_=corr32)
    nc.tensor.matmul(p0, ones16_1, corr16[:, 0:NHALF], start=False, stop=True)
    nc.tensor.matmul(p1, ones16_1, corr16[:, NHALF:NL], start=False, stop=True)

    # tail: risk = P + S_h ; partial = sum(theta*cen) - sum(ln(risk)*cen)
    lnt = tailp.tile([1, NL], DT.float32)
    nc.scalar.activation(out=lnt[:, 0:NHALF], in_=p0, func=AF.Ln, bias=sA)
    nc.scalar.activation(out=lnt[:, NHALF:NL], in_=p1, func=AF.Ln, bias=sA)
    lnc = tailp.tile([1, NL], DT.float32)
    nc.vector.tensor_mul(lnc, lnt, cenl)
    lc_sum = tailp.tile([1, 1], DT.float32)
    nc.vector.tensor_reduce(
        out=lc_sum, in_=lnc, axis=mybir.AxisListType.X, op=mybir.AluOpType.add
    )
    res = tailp.tile([1, 1], DT.float32)
    nc.vector.tensor_sub(res, thc_sum, lc_sum)
    nc.sync.dma_start(out=partial[:].rearrange("(o n) -> o n", o=1), in_=res)


def _build_nc(reps: int | None = None) -> bass.Bass:
    nc = bacc.Bacc()
    st32r_all = nc.declare_dram_parameter("st32r_all", [N], DT.float32, isOutput=False)
    th_all = nc.declare_dram_parameter("th_all", [N], DT.float32, isOutput=False)
    st16_loc = nc.declare_dram_parameter("st16_loc", [NL], DT.float16, isOutput=False)
    th_loc = nc.declare_dram_parameter("th_loc", [NL], DT.float32, isOutput=False)
    cen_loc = nc.declare_dram_parameter("cen_loc", [NL], DT.float32, isOutput=False)
    wv = nc.declare_dram_parameter("wv", [NL], DT.float32, isOutput=False)
    partial = nc.declare_dram_parameter("partial", [1], DT.float32, isOutput=True)

    with tile.TileContext(nc) as tc, ExitStack() as ctx:
        const = ctx.enter_context(tc.tile_pool(name="const", bufs=1))
        masks = ctx.enter_context(tc.tile_pool(name="masks", bufs=MASK_BUFS))
        psums = ctx.enter_context(tc.tile_pool(name="psums", bufs=1, space="PSUM"))
        tailp = ctx.enter_context(tc.tile_pool(name="tailp", bufs=1))

        loop = (
            tc.For_i(0, reps, 1,
                     hint_engines=(mybir.EngineType.PE, mybir.EngineType.DVE))
            if reps is not None
            else nullcontext()
        )
        with loop:
            _emit_body(nc, const, masks, psums, tailp, st32r_all, th_all,
                       st16_loc, th_loc, cen_loc, wv, partial)

    nc.compile()
    return nc


def _get_nc() -> bass.Bass:
    if "nc" not in _CACHE:
        _CACHE["nc"] = _build_nc()
    return _CACHE["nc"]


def make_in_maps(survtime: np.ndarray, theta: np.ndarray, censor: np.ndarray):
    st = np.ascontiguousarray(survtime, dtype=np.float32)
    st16 = st.astype(np.float16)
    st32r = st16.astype(np.float32)
    th = np.ascontiguousarray(theta, dtype=np.float32).reshape(-1)
    cen = np.ascontiguousarray(censor, dtype=np.float32)
    # wv[i] = 0.5 iff row i's chunk (i % 64) is an ACT Sign chunk
    _, act_lo = _splits()
    wv = np.where(np.arange(NL) % NCHUNK >= act_lo, 0.5, 0.0).astype(np.float32)
    in_maps = []
    for k in range(CORES):
        lo, hi = k * NL, (k + 1) * NL
        in_maps.append(
            {
                "st32r_all": st32r,
                "th_all": th,
                "st16_loc": st16[lo:hi].copy(),
                "th_loc": th[lo:hi].copy(),
                "cen_loc": cen[lo:hi].copy(),
                "wv": wv,
            }
        )
    return in_maps


def kernel(hazard_pred: np.ndarray, survtime: np.ndarray, censor: np.ndarray):
    nc = _get_nc()
    in_maps = make_in_maps(survtime, hazard_pred, censor)
    out = run_bass_kernel_spmd(nc, in_maps, list(range(CORES)))
    partials = np.array(
        [np.asarray(out.results[k]["partial"]).reshape(-1)[0] for k in range(CORES)],
        dtype=np.float64,
    )
    return np.float32(-partials.sum() / N)


# revision 9
# speedup vs baseline: 1.0052x; 1.0052x over previous
"""Cox partial-likelihood loss on 8 Trainium2 NeuronCores.

reference:
    theta = hazard_pred.reshape(-1)                 # [n]
    R[i, j] = survtime[j] >= survtime[i]            # risk-set mask
    risk_sum[i] = sum_j exp(theta[j]) * R[i, j]
    loss = -mean((theta - log(risk_sum)) * censor)

Sharding: rows i are split across 8 cores (1024 rows each). Each core
computes its [8192 x 1024] slice of e_j-weighted risk mask in 64 chunks
of 128 j's and contracts each chunk on the TensorEngine into PSUM.

v2 design (vs v1's DVE/ACT Sign-correction scheme):
  - survtime is cast to fp16 on the host; the DVE tensor_scalar
    (s_i <= s_j) * e_j runs with all-2-byte tensor operands, which
    engages the DVE 4x perf mode (~330ns/chunk vs 684 at 2x).
    Comparing fp16(s_i) <= fp16(s_j) keeps the diagonal exact, so no
    sign-correction machinery is needed at all; fp16 ties/rounding
    perturb the loss by ~1e-3 relative, well inside the 2e-2 gate.
  - The mask tile already carries e_j (scalar2 = per-partition fp32
    e column, exempt from the 2-byte rule), so every matmul uses the
    same constant ones[128,1] stationary vector: risk_sum[i] = P[i].
  - With mask production at ~330ns/chunk and PE consumption at
    ~432ns/chunk, the PE never starves, stays in continuous
    execution, and ramps to its 2.4 GHz p-state (the v1 kernel sat at
    the ~1.2 GHz mid p-state boundary, which is why it measured
    ~52-59us instead of its ~28us engine-busy floor).

j-index mapping: j = p*64 + c (p = SBUF partition, c = chunk column),
so survtime/theta load as contiguous [128, 64] tiles and chunk c uses
column c for the per-partition compare/weight scalars.

Host sums the 8 partial row-sums and applies -1/n.
"""

import sys
from contextlib import ExitStack, nullcontext

import numpy as np

try:  # concourse ships with the container toolchain, not on sys.path by default
    import concourse  # noqa: F401
except ImportError:
    sys.path.insert(0, "/opt/trn_rl_repo")

import concourse.bacc as bacc
import concourse.bass as bass
import concourse.tile as tile
from concourse import mybir
from concourse.bass_utils import run_bass_kernel_spmd

DT = mybir.dt
AF = mybir.ActivationFunctionType
N = 8192
CORES = 8
NL = N // CORES       # 1024 local rows per core
NCHUNK = 64           # j-chunks of 128
NHALF = NL // 2       # matmul free-dim limit is 512 (PSUM bank)

MASK_BUFS = 6
SIB_MODE = "hw4"  # 4-way HWDGE split broadcast

_CACHE: dict = {}


def _emit_body(nc, const, masks, psums, tailp, st32r_all, th_all, st16_loc,
               th_loc, cen_loc, partial):
    # j-major tiles: [p, c] holds index j = p*64 + c
    # st32r holds fp16-rounded survtime in fp32 (compare scalars must be
    # fp32); values match si16's fp16 rounding exactly, so the diagonal
    # i==j compare is a true tie and every row keeps its self-term.
    st32r = const.tile([128, NCHUNK], DT.float32)
    nc.sync.dma_start(out=st32r, in_=st32r_all[:].rearrange("(p c) -> p c", c=NCHUNK))
    th_sb = const.tile([128, NCHUNK], DT.float32)
    nc.sync.dma_start(out=th_sb, in_=th_all[:].rearrange("(p c) -> p c", c=NCHUNK))

    e32 = const.tile([128, NCHUNK], DT.float32)
    nc.scalar.activation(out=e32, in_=th_sb, func=AF.Exp)

    # tail inputs (DMAs early; dependent compute emitted after the loop)
    thl = tailp.tile([1, NL], DT.float32)
    nc.sync.dma_start(out=thl, in_=th_loc[:].rearrange("(o n) -> o n", o=1))
    cenl = tailp.tile([1, NL], DT.float32)
    nc.sync.dma_start(out=cenl, in_=cen_loc[:].rearrange("(o n) -> o n", o=1))

    # local survtime (fp16) broadcast to all partitions (free dim = row i)
    si16 = const.tile([128, NL], DT.float16)
    st_loc_row = st16_loc[:].rearrange("(o n) -> o n", o=1)
    for q in range(4):
        nc.sync.dma_start(
            out=si16[q * 32 : (q + 1) * 32, :],
            in_=st_loc_row.partition_broadcast(32),
        )

    ones16 = const.tile([128, 1], DT.float16)
    nc.vector.memset(ones16, 1.0)

    # main loop: P[i] accumulates sum_j e_j * (s_i <= s_j) via PE
    p0 = psums.tile([1, NHALF], DT.float32, tag="p0")
    p1 = psums.tile([1, NHALF], DT.float32, tag="p1")
    for c in range(NCHUNK):
        m = masks.tile([128, NL], DT.float16, tag="m")
        nc.vector.tensor_scalar(
            out=m,
            in0=si16,
            scalar1=st32r[:, c : c + 1],
            scalar2=e32[:, c : c + 1],
            op0=mybir.AluOpType.is_le,
            op1=mybir.AluOpType.mult,
        )
        nc.tensor.matmul(
            p0, ones16, m[:, 0:NHALF], start=(c == 0), stop=(c == NCHUNK - 1)
        )
        nc.tensor.matmul(
            p1, ones16, m[:, NHALF:NL], start=(c == 0), stop=(c == NCHUNK - 1)
        )

    # partial = sum(theta*censor) - sum(ln(risk)*censor); the theta*censor
    # term computes off the critical path while masks still run
    thc = tailp.tile([1, NL], DT.float32)
    nc.gpsimd.tensor_mul(thc, thl, cenl)
    thc_sum = tailp.tile([1, 1], DT.float32)
    nc.vector.tensor_reduce(
        out=thc_sum, in_=thc, axis=mybir.AxisListType.X, op=mybir.AluOpType.add
    )
    # dummy Ln pre-loads the Ln activation table while PE finishes the last
    # matmuls, so the real Ln isn't stalled on a ~1.3us table load
    onesf = tailp.tile([1, 1], DT.float32)
    nc.vector.memset(onesf, 1.0)
    ln_warm = tailp.tile([1, 1], DT.float32)
    nc.scalar.activation(out=ln_warm, in_=onesf, func=AF.Ln)

    # tail: risk = P
    lnt = tailp.tile([1, NL], DT.float32)
    nc.scalar.activation(out=lnt[:, 0:NHALF], in_=p0, func=AF.Ln)
    nc.scalar.activation(out=lnt[:, NHALF:NL], in_=p1, func=AF.Ln)
    lnc = tailp.tile([1, NL], DT.float32)
    nc.vector.tensor_mul(lnc, lnt, cenl)
    lc_sum = tailp.tile([1, 1], DT.float32)
    nc.vector.tensor_reduce(
        out=lc_sum, in_=lnc, axis=mybir.AxisListType.X, op=mybir.AluOpType.add
    )
    res = tailp.tile([1, 1], DT.float32)
    nc.vector.tensor_sub(res, thc_sum, lc_sum)
    nc.sync.dma_start(out=partial[:].rearrange("(o n) -> o n", o=1), in_=res)


def _build_nc(reps: int | None = None) -> bass.Bass:
    nc = bacc.Bacc()
    st32r_all = nc.declare_dram_parameter("st32r_all", [N], DT.float32, isOutput=False)
    th_all = nc.declare_dram_parameter("th_all", [N], DT.float32, isOutput=False)
    st16_loc = nc.declare_dram_parameter("st16_loc", [NL], DT.float16, isOutput=False)
    th_loc = nc.declare_dram_parameter("th_loc", [NL], DT.float32, isOutput=False)
    cen_loc = nc.declare_dram_parameter("cen_loc", [NL], DT.float32, isOutput=False)
    partial = nc.declare_dram_parameter("partial", [1], DT.float32, isOutput=True)

    with tile.TileContext(nc) as tc, ExitStack() as ctx:
        const = ctx.enter_context(tc.tile_pool(name="const", bufs=1))
        masks = ctx.enter_context(tc.tile_pool(name="masks", bufs=MASK_BUFS))
        psums = ctx.enter_context(tc.tile_pool(name="psums", bufs=1, space="PSUM"))
        tailp = ctx.enter_context(tc.tile_pool(name="tailp", bufs=1))

        loop = (
            tc.For_i(0, reps, 1,
                     hint_engines=(mybir.EngineType.PE, mybir.EngineType.DVE))
            if reps is not None
            else nullcontext()
        )
        with loop:
            _emit_body(nc, const, masks, psums, tailp, st32r_all, th_all,
                       st16_loc, th_loc, cen_loc, partial)

    nc.compile()
    return nc


def _get_nc() -> bass.Bass:
    if "nc" not in _CACHE:
        _CACHE["nc"] = _build_nc()
    return _CACHE["nc"]


def make_in_maps(survtime: np.ndarray, theta: np.ndarray, censor: np.ndarray):
    st = np.ascontiguousarray(survtime, dtype=np.float32)
    st16 = st.astype(np.float16)
    st32r = st16.astype(np.float32)
    th = np.ascontiguousarray(theta, dtype=np.float32).reshape(-1)
    cen = np.ascontiguousarray(censor, dtype=np.float32)
    in_maps = []
    for k in range(CORES):
        lo, hi = k * NL, (k + 1) * NL
        in_maps.append(
            {
                "st32r_all": st32r,
                "th_all": th,
                "st16_loc": st16[lo:hi].copy(),
                "th_loc": th[lo:hi].copy(),
                "cen_loc": cen[lo:hi].copy(),
            }
        )
    return in_maps


def kernel(hazard_pred: np.ndarray, survtime: np.ndarray, censor: np.ndarray):
    nc = _get_nc()
    in_maps = make_in_maps(survtime, hazard_pred, censor)
    out = run_bass_kernel_spmd(nc, in_maps, list(range(CORES)))
    partials = np.array(
        [np.asarray(out.results[k]["partial"]).reshape(-1)[0] for k in range(CORES)],
        dtype=np.float64,
    )
    return np.float32(-partials.sum() / N)


# revision 11
# speedup vs baseline: 1.2707x; 1.2641x over previous
"""Cox partial-likelihood loss on 8 Trainium2 NeuronCores.

reference:
    theta = hazard_pred.reshape(-1)                 # [n]
    R[i, j] = survtime[j] >= survtime[i]            # risk-set mask
    risk_sum[i] = sum_j exp(theta[j]) * R[i, j]
    loss = -mean((theta - log(risk_sum)) * censor)

Sharding: rows i are split across 8 cores (1024 rows each). Each core
computes its [8192 x 1024] slice of e_j-weighted risk mask in 64 chunks
of 128 j's and contracts each chunk on the TensorEngine into PSUM.

v2 design (vs v1's DVE/ACT Sign-correction scheme):
  - survtime is cast to fp16 on the host; the DVE tensor_scalar
    (s_i <= s_j) * e_j runs with all-2-byte tensor operands, which
    engages the DVE 4x perf mode (~330ns/chunk vs 684 at 2x).
    Comparing fp16(s_i) <= fp16(s_j) keeps the diagonal exact, so no
    sign-correction machinery is needed at all; fp16 ties/rounding
    perturb the loss by ~1e-3 relative, well inside the 2e-2 gate.
  - The mask tile already carries e_j (scalar2 = per-partition fp32
    e column, exempt from the 2-byte rule), so every matmul uses the
    same constant ones[128,1] stationary vector: risk_sum[i] = P[i].
  - With mask production at ~330ns/chunk and PE consumption at
    ~432ns/chunk, the PE never starves, stays in continuous
    execution, and ramps to its 2.4 GHz p-state (the v1 kernel sat at
    the ~1.2 GHz mid p-state boundary, which is why it measured
    ~52-59us instead of its ~28us engine-busy floor).

j-index mapping: j = p*64 + c (p = SBUF partition, c = chunk column),
so survtime/theta load as contiguous [128, 64] tiles and chunk c uses
column c for the per-partition compare/weight scalars.

Host sums the 8 partial row-sums and applies -1/n.
"""

import sys
from contextlib import ExitStack, nullcontext

import numpy as np

try:  # concourse ships with the container toolchain, not on sys.path by default
    import concourse  # noqa: F401
except ImportError:
    sys.path.insert(0, "/opt/trn_rl_repo")

import concourse.bacc as bacc
import concourse.bass as bass
import concourse.tile as tile
from concourse import mybir
from concourse.bass_utils import run_bass_kernel_spmd

DT = mybir.dt
AF = mybir.ActivationFunctionType
N = 8192
CORES = 8
NL = N // CORES       # 1024 local rows per core
NCHUNK = 64           # j-chunks of 128
NHALF = NL // 2       # matmul free-dim limit is 512 (PSUM bank)

MASK_BUFS = 10
SKEW = 4              # p0 trails p1 by SKEW chunks so half-1's Ln/reduce
                      # overlaps the last p0 matmuls
SIB_MODE = "hw4"  # 4-way HWDGE split broadcast

_CACHE: dict = {}


def _emit_body(nc, const, masks, psums, tailp, st32r_all, th_all, st16_loc,
               th_loc, cen_loc, partial):
    # j-major tiles: [p, c] holds index j = p*64 + c
    # st32r holds fp16-rounded survtime in fp32 (compare scalars must be
    # fp32); values match si16's fp16 rounding exactly, so the diagonal
    # i==j compare is a true tie and every row keeps its self-term.
    st32r = const.tile([128, NCHUNK], DT.float32)
    nc.sync.dma_start(out=st32r, in_=st32r_all[:].rearrange("(p c) -> p c", c=NCHUNK))
    th_sb = const.tile([128, NCHUNK], DT.float32)
    nc.sync.dma_start(out=th_sb, in_=th_all[:].rearrange("(p c) -> p c", c=NCHUNK))

    e32 = const.tile([128, NCHUNK], DT.float32)
    nc.scalar.activation(out=e32, in_=th_sb, func=AF.Exp)

    # tail inputs (DMAs early; dependent compute emitted after the loop)
    thl = tailp.tile([1, NL], DT.float32)
    nc.sync.dma_start(out=thl, in_=th_loc[:].rearrange("(o n) -> o n", o=1))
    cenl = tailp.tile([1, NL], DT.float32)
    nc.sync.dma_start(out=cenl, in_=cen_loc[:].rearrange("(o n) -> o n", o=1))

    # local survtime (fp16) broadcast to all partitions (free dim = row i)
    si16 = const.tile([128, NL], DT.float16)
    st_loc_row = st16_loc[:].rearrange("(o n) -> o n", o=1)
    for q in range(4):
        nc.sync.dma_start(
            out=si16[q * 32 : (q + 1) * 32, :],
            in_=st_loc_row.partition_broadcast(32),
        )

    ones16 = const.tile([128, 1], DT.float16)
    nc.vector.memset(ones16, 1.0)

    # dummy Ln pre-loads the Ln activation table early so the tail Ln
    # isn't stalled on a ~1.3us table load
    onesf = tailp.tile([1, 1], DT.float32)
    nc.vector.memset(onesf, 1.0)
    ln_warm = tailp.tile([1, 1], DT.float32)
    nc.scalar.activation(out=ln_warm, in_=onesf, func=AF.Ln)
    # theta*censor product off the critical path on gpsimd
    thc = tailp.tile([1, NL], DT.float32)
    nc.gpsimd.tensor_mul(thc, thl, cenl)

    # main loop: P[i] accumulates sum_j e_j * (s_i <= s_j) via PE.
    # p1 consumes chunk c at slot c; p0 trails by SKEW so p1 closes
    # early and its Ln+reduce overlap the final p0 matmuls.
    p0 = psums.tile([1, NHALF], DT.float32, tag="p0")
    p1 = psums.tile([1, NHALF], DT.float32, tag="p1")
    lnt = tailp.tile([1, NL], DT.float32)
    lnc = tailp.tile([1, NL], DT.float32)
    sum1 = tailp.tile([1, 1], DT.float32)
    tiles = {}
    for c in range(NCHUNK + SKEW):
        if c < NCHUNK:
            m = masks.tile([128, NL], DT.float16, tag="m")
            tiles[c] = m
            nc.vector.tensor_scalar(
                out=m,
                in0=si16,
                scalar1=st32r[:, c : c + 1],
                scalar2=e32[:, c : c + 1],
                op0=mybir.AluOpType.is_le,
                op1=mybir.AluOpType.mult,
            )
            nc.tensor.matmul(
                p1, ones16, m[:, NHALF:NL], start=(c == 0),
                stop=(c == NCHUNK - 1),
            )
        if c == NCHUNK:
            # p1 closed: start its tail while p0 finishes the last chunks
            nc.scalar.activation(out=lnt[:, NHALF:NL], in_=p1, func=AF.Ln)
            nc.vector.tensor_mul(lnc[:, NHALF:NL], lnt[:, NHALF:NL],
                                 cenl[:, NHALF:NL])
            nc.vector.tensor_reduce(
                out=sum1, in_=lnc[:, NHALF:NL], axis=mybir.AxisListType.X,
                op=mybir.AluOpType.add,
            )
        if c >= SKEW:
            cc = c - SKEW
            nc.tensor.matmul(
                p0, ones16, tiles.pop(cc)[:, 0:NHALF], start=(cc == 0),
                stop=(cc == NCHUNK - 1),
            )

    # tail: risk = P ; partial = sum(theta*cen) - sum(ln(risk)*cen)
    nc.scalar.activation(out=lnt[:, 0:NHALF], in_=p0, func=AF.Ln)
    lc_sum = tailp.tile([1, 1], DT.float32)
    nc.vector.tensor_mul(lnc[:, 0:NHALF], lnt[:, 0:NHALF], cenl[:, 0:NHALF])
    nc.vector.tensor_reduce(
        out=lc_sum, in_=lnc[:, 0:NHALF], axis=mybir.AxisListType.X,
        op=mybir.AluOpType.add,
    )
    thc_sum = tailp.tile([1, 1], DT.float32)
    nc.vector.tensor_reduce(
        out=thc_sum, in_=thc, axis=mybir.AxisListType.X, op=mybir.AluOpType.add
    )
    res = tailp.tile([1, 1], DT.float32)
    nc.vector.tensor_sub(res, thc_sum, lc_sum)
    nc.vector.tensor_sub(res, res, sum1)
    nc.sync.dma_start(out=partial[:].rearrange("(o n) -> o n", o=1), in_=res)


def _build_nc(reps: int | None = None) -> bass.Bass:
    nc = bacc.Bacc()
    st32r_all = nc.declare_dram_parameter("st32r_all", [N], DT.float32, isOutput=False)
    th_all = nc.declare_dram_parameter("th_all", [N], DT.float32, isOutput=False)
    st16_loc = nc.declare_dram_parameter("st16_loc", [NL], DT.float16, isOutput=False)
    th_loc = nc.declare_dram_parameter("th_loc", [NL], DT.float32, isOutput=False)
    cen_loc = nc.declare_dram_parameter("cen_loc", [NL], DT.float32, isOutput=False)
    partial = nc.declare_dram_parameter("partial", [1], DT.float32, isOutput=True)

    with tile.TileContext(nc) as tc, ExitStack() as ctx:
        const = ctx.enter_context(tc.tile_pool(name="const", bufs=1))
        masks = ctx.enter_context(tc.tile_pool(name="masks", bufs=MASK_BUFS))
        psums = ctx.enter_context(tc.tile_pool(name="psums", bufs=1, space="PSUM"))
        tailp = ctx.enter_context(tc.tile_pool(name="tailp", bufs=1))

        loop = (
            tc.For_i(0, reps, 1,
                     hint_engines=(mybir.EngineType.PE, mybir.EngineType.DVE))
            if reps is not None
            else nullcontext()
        )
        with loop:
            _emit_body(nc, const, masks, psums, tailp, st32r_all, th_all,
                       st16_loc, th_loc, cen_loc, partial)

    nc.compile()
    return nc


def _get_nc() -> bass.Bass:
    if "nc" not in _CACHE:
        _CACHE["nc"] = _build_nc()
    return _CACHE["nc"]


def make_in_maps(survtime: np.ndarray, theta: np.ndarray, censor: np.ndarray):
    st = np.ascontiguousarray(survtime, dtype=np.float32)
    st16 = st.astype(np.float16)
    st32r = st16.astype(np.float32)
    th = np.ascontiguousarray(theta, dtype=np.float32).reshape(-1)
    cen = np.ascontiguousarray(censor, dtype=np.float32)
    in_maps = []
    for k in range(CORES):
        lo, hi = k * NL, (k + 1) * NL
        in_maps.append(
            {
                "st32r_all": st32r,
                "th_all": th,
                "st16_loc": st16[lo:hi].copy(),
                "th_loc": th[lo:hi].copy(),
                "cen_loc": cen[lo:hi].copy(),
            }
        )
    return in_maps


def kernel(hazard_pred: np.ndarray, survtime: np.ndarray, censor: np.ndarray):
    nc = _get_nc()
    in_maps = make_in_maps(survtime, hazard_pred, censor)
    out = run_bass_kernel_spmd(nc, in_maps, list(range(CORES)))
    partials = np.array(
        [np.asarray(out.results[k]["partial"]).reshape(-1)[0] for k in range(CORES)],
        dtype=np.float64,
    )
    return np.float32(-partials.sum() / N)
